# revision 1
# baseline (speedup 1.0000x reference)
"""Trainium2 Bass kernel for nn_MEGNet_State_876173328941.

MEGNet state update: u_e = scatter_mean(edge_attr, batch[edge_index[0]], B),
u_v = scatter_mean(x, batch, B), comb = [u_e, u_v, state], then a 3-layer MLP
(96->32->32->32) with training-mode BatchNorm over the batch dim.

Sharding strategy (host side, inside kernel()):
  - The 1024 graphs are assigned to the 8 cores with a balanced (LPT)
    partition of their edge-tile counts; each core owns 128 graphs. Within a
    core, graphs are ranked by size; slot i's tile count (sched_e[i]) is the
    max over cores at that rank, so all cores share ONE SPMD program. Rows
    are zero-padded into their slots with a 33rd "ones" column marking real
    rows (the device computes per-graph counts itself).
  - Device: each 128-row tile is reduced with one TensorE matmul
    (lhsT = rows [128, 33], rhs = ones [128, 1]) accumulating straight into
    PSUM column i of a per-core [33, 129] segment-sum accumulator
    (column 128 is a scratch column for pad tiles).
  - Per-core partial results are AllGathered; every core then computes the
    scatter-mean division and the tiny MLP with BatchNorm redundantly in
    transposed layout [feat, graph]. Host takes core 0's output and undoes
    the graph permutation.
"""

import sys

sys.path.insert(0, "/opt/trn_rl_repo")

import numpy as np

import concourse.bacc as bacc
import concourse.tile as tile
from concourse import mybir
from concourse.bass_utils import run_bass_kernel_spmd

DIM = 32
DIMC = DIM + 1      # +1 ones column for counts
B = 1024
N_CORES = 8
SEGS = 128          # graphs per core
CH = 128            # tiles per DMA chunk
EPS = 1e-5
AGR = 128           # allgather rows: 0-31 e-sums, 32-63 v-sums, 64 e-cnt, 96 v-cnt

_CACHE = {}


def _plan(ecnt, ncnt):
    """Balanced graph->core assignment plus shared per-rank slot schedule."""
    e_tiles = np.maximum((ecnt + 127) // 128, 1).astype(np.int64)
    n_tiles = np.maximum((ncnt + 127) // 128, 1).astype(np.int64)

    order_desc = np.argsort(-e_tiles, kind="stable")
    load = np.zeros(N_CORES, dtype=np.int64)
    nseg = np.zeros(N_CORES, dtype=np.int64)
    assign = np.zeros(B, dtype=np.int64)
    for s in order_desc:
        open_cores = np.where(nseg < SEGS)[0]
        k = open_cores[np.argmin(load[open_cores])]
        assign[s] = k
        load[k] += e_tiles[s]
        nseg[k] += 1

    # per-core rank order: this core's graphs sorted by e_tiles desc
    order = np.zeros((N_CORES, SEGS), dtype=np.int64)   # rank -> global seg
    rank_of = np.zeros(B, dtype=np.int64)
    for k in range(N_CORES):
        segs_k = np.where(assign == k)[0]
        segs_k = segs_k[np.argsort(-e_tiles[segs_k], kind="stable")]
        order[k] = segs_k
        rank_of[segs_k] = np.arange(SEGS)

    sched_e = e_tiles[order].max(axis=0)   # [SEGS]
    sched_n = n_tiles[order].max(axis=0)   # [SEGS]
    p_global = order.reshape(-1)           # gathered col j -> global seg
    return assign, rank_of, sched_e, sched_n, p_global


def _tile_plan(sched):
    """[(col, start, stop)] per tile, padded to a CH multiple with scratch."""
    plan = []
    for i, t in enumerate(sched):
        for j in range(int(t)):
            plan.append((i, j == 0, j == int(t) - 1))
    while len(plan) % CH:
        plan.append((SEGS, True, True))   # scratch column
    return plan


def _build_nc(plan_e, plan_n):
    nc = bacc.Bacc("TRN2", target_bir_lowering=False, debug=False,
                   enable_asserts=False, num_devices=N_CORES)
    f32 = mybir.dt.float32

    ev_chunks = len(plan_e) // CH
    nv_chunks = len(plan_n) // CH
    ev = nc.declare_dram_parameter("ev", [ev_chunks, 128, CH * DIMC], f32, isOutput=False)
    nv = nc.declare_dram_parameter("nv", [nv_chunks, 128, CH * DIMC], f32, isOutput=False)
    stateT = nc.declare_dram_parameter("stateT", [DIM, B], f32, isOutput=False)
    W1 = nc.declare_dram_parameter("W1", [3 * DIM, DIM], f32, isOutput=False)
    W2 = nc.declare_dram_parameter("W2", [DIM, DIM], f32, isOutput=False)
    W3 = nc.declare_dram_parameter("W3", [DIM, DIM], f32, isOutput=False)
    # vecs columns: b1,g1,be1,b2,g2,be2,b3,g3,be3
    vecs = nc.declare_dram_parameter("vecs", [DIM, 9], f32, isOutput=False)
    out = nc.declare_dram_parameter("out", [DIM, B], f32, isOutput=True)

    ag_in = nc.dram_tensor("ag_in", [AGR, SEGS], f32)
    ag_out = nc.dram_tensor("ag_out", [AGR * N_CORES, SEGS], f32,
                            addr_space="Shared")

    with tile.TileContext(nc) as tc:
        with tc.tile_pool(name="chunks", bufs=3) as chunks, \
             tc.tile_pool(name="const", bufs=1) as const, \
             tc.tile_pool(name="work", bufs=1) as work, \
             tc.tile_pool(name="spsum", bufs=1, space="PSUM") as spsum, \
             tc.tile_pool(name="mpsum", bufs=1, space="PSUM") as mpsum:

            ones = const.tile([128, 1], f32)
            nc.vector.memset(ones, 1.0)
            onesP = const.tile([128, DIM], f32)
            nc.vector.memset(onesP, 1.0)

            # ---- stage 1: streamed per-graph segment sums ----
            ps_e = spsum.tile([DIMC, SEGS + 1], f32, tag="ps_e")
            ps_n = spsum.tile([DIMC, SEGS + 1], f32, tag="ps_n")

            def stream(param, plan, psum_tile):
                n_chunks = len(plan) // CH
                for c in range(n_chunks):
                    ct = chunks.tile([128, CH * DIMC], f32, tag="chunk")
                    nc.sync.dma_start(out=ct, in_=param[c])
                    for t in range(CH):
                        col, start, stop = plan[c * CH + t]
                        nc.tensor.matmul(
                            out=psum_tile[:, col:col + 1],
                            lhsT=ct[:, t * DIMC:(t + 1) * DIMC],
                            rhs=ones[:, :],
                            start=start,
                            stop=stop,
                        )

            stream(ev, plan_e, ps_e)
            stream(nv, plan_n, ps_n)

            sums_e = work.tile([DIMC, SEGS], f32, tag="sums_e")
            nc.vector.tensor_copy(sums_e, ps_e[:, 0:SEGS])
            sums_n = work.tile([DIMC, SEGS], f32, tag="sums_n")
            nc.vector.tensor_copy(sums_n, ps_n[:, 0:SEGS])

            # ---- collective: gather all cores' slices ----
            zrows = const.tile([128, SEGS], f32)
            nc.vector.memset(zrows, 0.0)
            nc.sync.dma_start(out=ag_in[:, :], in_=zrows)
            nc.sync.dma_start(out=ag_in[0:DIM, :], in_=sums_e[0:DIM, :])
            nc.sync.dma_start(out=ag_in[DIM:2 * DIM, :], in_=sums_n[0:DIM, :])
            nc.sync.dma_start(out=ag_in[64:65, :], in_=sums_e[DIM:DIMC, :])
            nc.sync.dma_start(out=ag_in[96:97, :], in_=sums_n[DIM:DIMC, :])
            nc.gpsimd.collective_compute(
                "AllGather",
                mybir.AluOpType.bypass,
                replica_groups=[list(range(N_CORES))],
                ins=[ag_in[:, :]],
                outs=[ag_out[:, :]],
            )
            full = work.tile([AGR, B], f32, tag="full")
            agv = ag_out.rearrange("(r p) s -> r p s", p=AGR)
            for r in range(N_CORES):
                nc.sync.dma_start(out=full[:, r * SEGS:(r + 1) * SEGS], in_=agv[r])

            # ---- scatter-mean division ----
            rec = work.tile([AGR, B], f32, tag="rec")
            nc.vector.tensor_scalar_max(rec[64:97, :], full[64:97, :], 1.0)
            nc.vector.reciprocal(rec[64:97, :], rec[64:97, :])

            # broadcast recip rows across DIM partitions via matmul
            pb = mpsum.tile([2 * DIM, B], f32, tag="pb")
            for half in range(2):
                sl = slice(half * 512, (half + 1) * 512)
                nc.tensor.matmul(out=pb[0:DIM, sl], lhsT=onesP[64:65, :],
                                 rhs=rec[64:65, sl], start=True, stop=True,
                                 tile_position=(64, 0))
                nc.tensor.matmul(out=pb[DIM:2 * DIM, sl], lhsT=onesP[96:97, :],
                                 rhs=rec[96:97, sl], start=True, stop=True,
                                 tile_position=(96, 32))

            comb = work.tile([3 * DIM, B], f32, tag="comb")
            nc.vector.tensor_tensor(comb[0:DIM, :], full[0:DIM, :],
                                    pb[0:DIM, :], mybir.AluOpType.mult)
            nc.vector.tensor_tensor(comb[DIM:2 * DIM, :], full[DIM:2 * DIM, :],
                                    pb[DIM:2 * DIM, :], mybir.AluOpType.mult)
            nc.sync.dma_start(out=comb[2 * DIM:3 * DIM, :], in_=stateT[:, :])

            # ---- MLP with BatchNorm (transposed layout [feat, graph]) ----
            w1s = const.tile([3 * DIM, DIM], f32)
            nc.sync.dma_start(out=w1s, in_=W1[:, :])
            w2s = const.tile([DIM, DIM], f32)
            nc.sync.dma_start(out=w2s, in_=W2[:, :])
            w3s = const.tile([DIM, DIM], f32)
            nc.sync.dma_start(out=w3s, in_=W3[:, :])
            vs = const.tile([DIM, 9], f32)
            nc.sync.dma_start(out=vs, in_=vecs[:, :])

            h = comb
            for layer in range(3):
                w = (w1s, w2s, w3s)[layer]
                bcol = vs[:, 3 * layer:3 * layer + 1]
                gcol = vs[:, 3 * layer + 1:3 * layer + 2]
                becol = vs[:, 3 * layer + 2:3 * layer + 3]

                ps_h = mpsum.tile([DIM, B], f32, tag="ps_h")
                for half in range(2):
                    sl = slice(half * 512, (half + 1) * 512)
                    nc.tensor.matmul(out=ps_h[:, sl], lhsT=w[:, :], rhs=h[:, sl],
                                     start=True, stop=True)
                hl = work.tile([DIM, B], f32, tag=f"h{layer}")
                func = (mybir.ActivationFunctionType.Relu if layer < 2
                        else mybir.ActivationFunctionType.Identity)
                nc.scalar.activation(out=hl, in_=ps_h, func=func, bias=bcol)

                # batchnorm over the free (graph) dim
                msum = work.tile([DIM, 1], f32, tag="msum")
                nc.vector.tensor_reduce(out=msum, in_=hl,
                                        axis=mybir.AxisListType.X,
                                        op=mybir.AluOpType.add)
                m = work.tile([DIM, 1], f32, tag="m")
                nc.scalar.mul(m, msum, 1.0 / B)
                hc = work.tile([DIM, B], f32, tag=f"hc{layer}")
                nc.vector.tensor_scalar(hc, hl, m, None,
                                        mybir.AluOpType.subtract)
                sq = work.tile([DIM, B], f32, tag="sq")
                vsum = work.tile([DIM, 1], f32, tag="vsum")
                nc.scalar.activation(out=sq, in_=hc,
                                     func=mybir.ActivationFunctionType.Square,
                                     accum_out=vsum)
                veps = work.tile([DIM, 1], f32, tag="veps")
                nc.scalar.activation(out=veps, in_=vsum,
                                     func=mybir.ActivationFunctionType.Copy,
                                     bias=EPS, scale=1.0 / B)
                sd = work.tile([DIM, 1], f32, tag="sd")
                nc.scalar.sqrt(sd, veps)
                rstd = work.tile([DIM, 1], f32, tag="rstd")
                nc.vector.reciprocal(rstd, sd)
                rg = work.tile([DIM, 1], f32, tag="rg")
                nc.vector.tensor_tensor(rg, rstd, gcol, mybir.AluOpType.mult)
                hb = work.tile([DIM, B], f32, tag=f"hb{layer}")
                nc.vector.tensor_scalar(hb, hc, rg, becol,
                                        mybir.AluOpType.mult,
                                        mybir.AluOpType.add)
                h = hb

            nc.sync.dma_start(out=out[:, :], in_=h)

    nc.compile()
    return nc


def _pack(rows, seg, cnt, assign, rank_of, sched):
    """Scatter rows (f32 [M, 33], ones col included) into per-core DMA layout
    [N_CORES, n_chunks, 128, CH*33] per the shared slot schedule."""
    M = rows.shape[0]
    base = np.zeros(SEGS + 1, dtype=np.int64)
    np.cumsum(sched, out=base[1:])            # slot base tile per rank
    total_tiles = int(base[-1])
    n_chunks = (total_tiles + CH - 1) // CH
    pad_tiles = n_chunks * CH

    order = np.argsort(seg, kind="stable")
    srows = rows[order]
    sseg = seg[order]
    offs = np.zeros(B, dtype=np.int64)
    np.cumsum(cnt[:-1], out=offs[1:])
    within = np.arange(M, dtype=np.int64) - offs[sseg]

    core = assign[sseg]
    rank = rank_of[sseg]
    g = base[rank] + (within >> 7)            # tile within core
    c, t, p = g // CH, g % CH, within & 127
    P = np.zeros((N_CORES, n_chunks, 128, CH, DIMC), dtype=np.float32)
    P[core, c, p, t] = srows
    return P.reshape(N_CORES, n_chunks, 128, CH * DIMC)


def run(inputs, trace=False, sim=False):
    x = np.asarray(inputs["x"], dtype=np.float32)
    edge_index = np.asarray(inputs["edge_index"]).astype(np.int64)
    edge_attr = np.asarray(inputs["edge_attr"], dtype=np.float32)
    state = np.asarray(inputs["state"], dtype=np.float32)
    batch = np.asarray(inputs["batch"]).astype(np.int64)

    E = edge_attr.shape[0]
    N = x.shape[0]
    eseg = batch[edge_index[0]]
    ecnt = np.bincount(eseg, minlength=B)
    ncnt = np.bincount(batch, minlength=B)

    assign, rank_of, sched_e, sched_n, p_global = _plan(ecnt, ncnt)
    plan_e = _tile_plan(sched_e)
    plan_n = _tile_plan(sched_n)

    erows = np.empty((E, DIMC), dtype=np.float32)
    erows[:, :DIM] = edge_attr
    erows[:, DIM] = 1.0
    nrows = np.empty((N, DIMC), dtype=np.float32)
    nrows[:, :DIM] = x
    nrows[:, DIM] = 1.0

    ev = _pack(erows, eseg, ecnt, assign, rank_of, sched_e)
    nv = _pack(nrows, batch, ncnt, assign, rank_of, sched_n)

    vecs = np.stack([np.asarray(inputs[k], np.float32) for k in
                     ("b1", "g1", "be1", "b2", "g2", "be2", "b3", "g3", "be3")],
                    axis=1).astype(np.float32)  # [32, 9]

    shared = {
        "stateT": np.ascontiguousarray(state.T[:, p_global]),
        "W1": np.asarray(inputs["W1"], np.float32),
        "W2": np.asarray(inputs["W2"], np.float32),
        "W3": np.asarray(inputs["W3"], np.float32),
        "vecs": vecs,
    }
    in_maps = []
    for k in range(N_CORES):
        m = dict(shared)
        m["ev"] = np.ascontiguousarray(ev[k])
        m["nv"] = np.ascontiguousarray(nv[k])
        in_maps.append(m)

    key = (tuple(sched_e), tuple(sched_n))
    if key not in _CACHE:
        _CACHE[key] = _build_nc(plan_e, plan_n)
    nc = _CACHE[key]

    if sim:
        from concourse.bass_interp import MultiCoreSim
        msim = MultiCoreSim(nc, num_cores=N_CORES)
        for c in range(N_CORES):
            cs = msim.cores[c]
            for kk, vv in in_maps[c].items():
                cs.tensor(kk)[:] = vv
        msim.simulate(check_with_hw=False)
        outT = np.array(msim.cores[0].tensor("out"))
        res = None
    else:
        res = run_bass_kernel_spmd(nc, in_maps, core_ids=list(range(N_CORES)),
                                   trace=trace)
        outT = res.results[0]["out"]  # [32, 1024] in permuted graph order

    outP = outT.T.astype(np.float32)          # [1024(perm), 32]
    outF = np.empty_like(outP)
    outF[p_global] = outP
    return np.ascontiguousarray(outF), res


def kernel(**inputs) -> np.ndarray:
    out, _ = run(inputs, trace=False)
    return out

